# revision 14
# baseline (speedup 1.0000x reference)
"""Trainium2 Bass kernel for nn_LoRAPool (MoE top-2 LoRA expert pool).

Math (reference):
    gates[t,e] = p_L[t,e] if e in top-2 of p_L[t,:] else 0
    hr[t,e,r]  = sum_d h[t,d] * A[e,r,d]
    out[t,d]   = sum_{e,r} hr[t,e,r] * 2.0 * gates[t,e] * B[e,d,r]

Folded into two dense matmuls over c = (e,r) in [0,128):
    A_cat[d,c] = 2.0 * A[e,r,d];  B_cat[c,d] = B[e,d,r]
    U^T[c,t]   = sum_d A_cat[d,c] h[t,d]        (stage 1, PE)
    Us[c,t]    = U^T[c,t] * gates[t, c//16]     (gating, DVE)
    out[t,d]   = sum_c Us[c,t] B_cat[c,d]       (stage 2, PE)

Layout/scheduling tricks:
  - h is fed in bf16 and transposed during the HBM->SBUF DMA by the
    XBAR engine (InstDmaTransposeAnt): ht[c,j,t] = h[t, 128j+c]. The PE
    never transposes h.
  - The tile framework issues DMAs through a small global in-flight
    window, so the kernel uses FEW, LARGE DMA instructions (12 total:
    3 consts + 1 p_L + 4 group transposes + 4 group outputs) to keep
    HBM busy despite the window. The big A/B const loads first so the
    transpose backbone isn't gated on it late.
  - Gate blocks for all groups run first (they only need p_L), then the
    stage-1/stage-2 pipeline is software-pipelined one group apart so
    the PE never waits for the DVE gating multiply. Stage-2 uses four
    single-bank PSUM buffers with casts alternating DVE/ACT so the PE
    never stalls on PSUM drain.
  - All matmuls in bf16 (1 cycle/row); top-2 selection in f32 (exact
    expert choice); I/O in bf16 halves HBM traffic.

Sharding: tokens (4*4096 = 16384) split evenly across 8 cores; A/B and
the small expert-expand matrix are replicated.
"""

import numpy as np
import ml_dtypes

N_CORES = 8
B_SZ, S_SZ, D = 4, 4096, 2048
E, R, C = 8, 16, 128
T_FULL = B_SZ * S_SZ            # 16384 tokens
T_CORE = T_FULL // N_CORES      # 2048 tokens per core
GROUP = 512                     # token group (matmul moving dim)
N_GROUPS = T_CORE // GROUP      # 4
N_SUB = GROUP // 128            # 4 sub-tiles of 128 tokens
KD = D // 128                   # 16 contraction chunks
SCALING = 2.0

_CACHE = {}


def _build_nc(split_waits=True):
    import concourse.bass as bass
    import concourse.tile as tile
    import concourse.mybir as mybir
    from contextlib import ExitStack

    f32 = mybir.dt.float32
    f32r = mybir.dt.float32r
    bf16 = mybir.dt.bfloat16

    nc = bass.Bass()
    h_d = nc.declare_dram_parameter("h", [T_CORE, D], bf16, isOutput=False)
    p_d = nc.declare_dram_parameter("p_L", [T_CORE, E], f32, isOutput=False)
    ab_d = nc.declare_dram_parameter("AB", [128, 2 * KD * 128], bf16, isOutput=False)
    m_d = nc.declare_dram_parameter("Mexp", [E, C], f32r, isOutput=False)
    i_d = nc.declare_dram_parameter("Ident", [128, 128], f32, isOutput=False)
    o_d = nc.declare_dram_parameter("out", [T_CORE, D], bf16, isOutput=True)

    AX = mybir.AxisListType
    OP = mybir.AluOpType

    with ExitStack() as ctx:
        tc = ctx.enter_context(tile.TileContext(nc))
        consts = ctx.enter_context(tc.tile_pool(name="consts", bufs=1))
        htpool = ctx.enter_context(tc.tile_pool(name="ht", bufs=1))
        gpool = ctx.enter_context(tc.tile_pool(name="gates", bufs=2))
        gsbpool = ctx.enter_context(tc.tile_pool(name="gsb", bufs=N_GROUPS))
        utspool = ctx.enter_context(tc.tile_pool(name="uts", bufs=2))
        outpool = ctx.enter_context(tc.tile_pool(name="osb", bufs=2))
        ps_u = ctx.enter_context(tc.tile_pool(name="ps_u", bufs=2, space="PSUM"))
        ps_g = ctx.enter_context(tc.tile_pool(name="ps_g", bufs=2, space="PSUM"))
        ps_o = ctx.enter_context(tc.tile_pool(name="ps_o", bufs=4, space="PSUM"))

        # ---- few, large DMAs: p on GpSimd (its only DMA); consts then
        # transposes on SP. Last 512-group split in two 256-token pieces
        # to shrink the post-DMA tail.
        p_all = consts.tile([128, N_GROUPS * N_SUB, E], f32)
        nc.gpsimd.dma_start(
            out=p_all, in_=p_d[:, :].rearrange("(q p) e -> p q e", p=128)
        )
        AB_sb = consts.tile([128, 2 * KD * 128], bf16)
        nc.sync.dma_start(out=AB_sb, in_=ab_d[:, :])
        M_sb = consts.tile([E, C], f32r)
        nc.sync.dma_start(out=M_sb, in_=m_d[:, :])
        I_sb = consts.tile([128, 128], f32)
        nc.sync.dma_start(out=I_sb, in_=i_d[:, :])
        A_sb = AB_sb[:, : KD * 128].rearrange("p (k c) -> p k c", k=KD)
        B_sb = AB_sb[:, KD * 128 :]

        # Transposes alternate between the two HWDGE queues: descriptor
        # generation (~8us per 2MB group) runs in parallel on ACT and SP.
        PIECES = [(0, 512), (512, 512), (1024, 512), (1536, 512)]
        ht = []
        for i, (t0, sz) in enumerate(PIECES):
            hth = htpool.tile([128, KD, sz], bf16, tag=f"ht{t0}")
            eng = nc.scalar if i % 2 == 0 else nc.sync
            eng.dma_start_transpose(out=hth, in_=h_d[t0 : t0 + sz, :])
            ht.append(hth)

        # ---- gate blocks for every group (only depend on p_L) ----
        G_tiles = []
        for g in range(N_GROUPS):
            p_sb = p_all[:, g * N_SUB : (g + 1) * N_SUB, :]
            m1 = gpool.tile([128, N_SUB, 1], f32, tag="m1")
            nc.vector.tensor_reduce(out=m1, in_=p_sb, axis=AX.X, op=OP.max)
            mlt = gpool.tile([128, N_SUB, E], f32, tag="mlt")
            nc.vector.tensor_tensor(
                out=mlt, in0=p_sb, in1=m1.broadcast_to([128, N_SUB, E]), op=OP.is_lt
            )
            pm = gpool.tile([128, N_SUB, E], f32, tag="pm")
            nc.vector.tensor_mul(pm, p_sb, mlt)
            m2 = gpool.tile([128, N_SUB, 1], f32, tag="m2")
            nc.vector.tensor_reduce(out=m2, in_=pm, axis=AX.X, op=OP.max)
            ge2 = gpool.tile([128, N_SUB, E], f32, tag="ge2")
            nc.vector.tensor_tensor(
                out=ge2, in0=p_sb, in1=m2.broadcast_to([128, N_SUB, E]), op=OP.is_ge
            )
            gts = gpool.tile([128, N_SUB, E], f32, tag="gts")
            nc.vector.tensor_mul(gts, p_sb, ge2)

            gt_ps = ps_g.tile([128, GROUP], f32, tag="g")
            for s in range(N_SUB):
                nc.tensor.transpose(
                    out=gt_ps[:E, s * 128 : (s + 1) * 128],
                    in_=gts[:, s, :],
                    identity=I_sb,
                )
            gt_sb = gpool.tile([E, GROUP], f32r, tag="gtsb")
            nc.scalar.copy(out=gt_sb, in_=gt_ps[:E, :])
            G_ps = ps_g.tile([128, GROUP], f32, tag="g")
            nc.tensor.matmul(G_ps, lhsT=M_sb, rhs=gt_sb, start=True, stop=True)
            G_sb = gsbpool.tile([128, GROUP], f32, tag="gsb")
            nc.scalar.copy(out=G_sb, in_=G_ps)
            G_tiles.append(G_sb)

        # ---- software-pipelined stage 1 / stage 2 (one piece apart) ----
        uts_tiles = [None] * len(PIECES)

        def stage1(i):
            t0, sz = PIECES[i]
            U_ps = ps_u.tile([128, GROUP], f32, tag="u")
            for k in range(KD):
                nc.tensor.matmul(
                    U_ps[:, :sz],
                    lhsT=A_sb[:, k, :],
                    rhs=ht[i][:, k, :],
                    start=(k == 0),
                    stop=(k == KD - 1),
                )
            uts = utspool.tile([128, GROUP], bf16, tag="uts")
            g, off = t0 // GROUP, t0 % GROUP
            nc.vector.tensor_tensor(
                out=uts[:, :sz],
                in0=U_ps[:, :sz],
                in1=G_tiles[g][:, off : off + sz],
                op=OP.mult,
            )
            uts_tiles[i] = uts

        def stage2(i):
            t0, sz = PIECES[i]
            uts = uts_tiles[i]
            o_sb = outpool.tile([128, N_SUB, D], bf16, tag="osb")
            for s in range(sz // 128):
                for j in range(4):
                    o_ps = ps_o.tile([128, 512], f32, tag="ops")
                    nc.tensor.matmul(
                        o_ps,
                        lhsT=uts[:, s * 128 : (s + 1) * 128],
                        rhs=B_sb[:, j * 512 : (j + 1) * 512],
                        start=True,
                        stop=True,
                    )
                    dst = o_sb[:, s, j * 512 : (j + 1) * 512]
                    if j % 2 == 0:
                        nc.vector.tensor_copy(out=dst, in_=o_ps)
                    else:
                        nc.scalar.copy(out=dst, in_=o_ps)
            nc.sync.dma_start(
                out=o_d[t0 : t0 + sz, :].rearrange("(s p) d -> p s d", p=128),
                in_=o_sb[:, : sz // 128, :],
            )

        for i in range(len(PIECES)):
            stage1(i)
            if i >= 1:
                stage2(i - 1)
        stage2(len(PIECES) - 1)

    if split_waits:
        _split_matmul_waits(nc)
    return nc


def _split_matmul_waits(nc, max_waits=1):
    """Walrus codegen allows only one sync-wait slot on self-loading
    (fp32/fp32r) Matmult instructions. Move surplus waits onto a no-op
    EventSemaphore inserted immediately before, same engine — identical
    semantics (waits still complete before the matmul dispatches)."""
    import concourse.mybir as mybir

    n = 0
    for f in nc.m.functions:
        for blk in f.blocks:
            insts = blk.instructions
            new_list = []
            changed = False
            for inst in insts:
                si = inst.sync_info
                if (
                    type(inst).__name__ != "InstEventSemaphore"
                    and si is not None
                    and si.on_wait
                    and len(si.on_wait) > max_waits
                ):
                    surplus = list(si.on_wait[:-max_waits])
                    keep = list(si.on_wait[-max_waits:])
                    # EventSemaphore carriers take at most 2 waits each
                    for i in range(0, len(surplus), 2):
                        n += 1
                        ev = mybir.InstEventSemaphore(
                            name=f"I-swsplit-{n}", ins=[], outs=[]
                        )
                        ev.engine = inst.engine
                        ev.sync_info = mybir.SyncInfo(
                            on_wait=surplus[i : i + 2], on_update=[]
                        )
                        new_list.append(ev)
                    inst.sync_info = mybir.SyncInfo(
                        on_wait=keep, on_update=list(si.on_update or [])
                    )
                    changed = True
                new_list.append(inst)
            if changed:
                blk.instructions = new_list
    return n


def _host_prep(h, p_L, A, B):
    """Shard tokens across cores; build replicated helper matrices."""
    bf16 = ml_dtypes.bfloat16
    h_flat = np.ascontiguousarray(
        np.asarray(h, dtype=np.float32).reshape(T_FULL, D)
    ).astype(bf16)
    p_flat = np.ascontiguousarray(np.asarray(p_L, dtype=np.float32).reshape(T_FULL, E))
    # A_cat[d, c] = SCALING * A[e, r, d]
    A_cat = (np.asarray(A, dtype=np.float32) * SCALING).transpose(2, 0, 1).reshape(D, C)
    # XBAR layout: ht[c, k, t] = h[t, 128k + c]  =>  A3[c, k, :] = A_cat[128k+c, :]
    A3 = np.ascontiguousarray(
        A_cat.reshape(KD, 128, C).transpose(1, 0, 2)
    ).astype(bf16)
    # B_cat[c, d] = B[e, d, r]
    B_cat = np.ascontiguousarray(
        np.asarray(B, dtype=np.float32).transpose(0, 2, 1).reshape(C, D)
    ).astype(bf16)
    AB = np.ascontiguousarray(
        np.concatenate([A3.reshape(128, KD * 128), B_cat], axis=1)
    )
    Mexp = np.zeros((E, C), dtype=np.float32)
    for e in range(E):
        Mexp[e, e * R : (e + 1) * R] = 1.0
    Ident = np.eye(128, dtype=np.float32)
    in_maps = []
    for i in range(N_CORES):
        sl = slice(i * T_CORE, (i + 1) * T_CORE)
        in_maps.append(
            {
                "h": h_flat[sl],
                "p_L": p_flat[sl],
                "AB": AB,
                "Mexp": Mexp,
                "Ident": Ident,
            }
        )
    return in_maps


def _get_nc():
    if "nc" not in _CACHE:
        _CACHE["nc"] = _build_nc()
    return _CACHE["nc"]


def kernel(h, p_L, A, B):
    from concourse.bass_utils import run_bass_kernel_spmd

    nc = _get_nc()
    in_maps = _host_prep(h, p_L, A, B)
    res = run_bass_kernel_spmd(nc, in_maps, core_ids=list(range(N_CORES)))
    out = np.concatenate(
        [np.asarray(res.results[i]["out"]) for i in range(N_CORES)], axis=0
    )
    return out.astype(np.float32).reshape(B_SZ, S_SZ, D)


# revision 16
# speedup vs baseline: 1.1544x; 1.1544x over previous
"""Trainium2 Bass kernel for nn_LoRAPool (MoE top-2 LoRA expert pool).

Math (reference):
    gates[t,e] = p_L[t,e] if e in top-2 of p_L[t,:] else 0
    hr[t,e,r]  = sum_d h[t,d] * A[e,r,d]
    out[t,d]   = sum_{e,r} hr[t,e,r] * 2.0 * gates[t,e] * B[e,d,r]

Folded into two dense matmuls over c = (e,r) in [0,128):
    A_cat[d,c] = 2.0 * A[e,r,d];  B_cat[c,d] = B[e,d,r]
    U^T[c,t]   = sum_d A_cat[d,c] h[t,d]        (stage 1, PE)
    Us[c,t]    = U^T[c,t] * gates[t, c//16]     (gating, DVE)
    out[t,d]   = sum_c Us[c,t] B_cat[c,d]       (stage 2, PE)

Layout/scheduling tricks:
  - h is fed in bf16 and transposed during the HBM->SBUF DMA by the
    XBAR engine (InstDmaTransposeAnt): ht[c,j,t] = h[t, 128j+c]. The PE
    never transposes h.
  - The tile framework issues DMAs through a small global in-flight
    window, so the kernel uses FEW, LARGE DMA instructions (12 total:
    3 consts + 1 p_L + 4 group transposes + 4 group outputs) to keep
    HBM busy despite the window. The big A/B const loads first so the
    transpose backbone isn't gated on it late.
  - Gate blocks for all groups run first (they only need p_L), then the
    stage-1/stage-2 pipeline is software-pipelined one group apart so
    the PE never waits for the DVE gating multiply. Stage-2 uses four
    single-bank PSUM buffers with casts alternating DVE/ACT so the PE
    never stalls on PSUM drain.
  - All matmuls in bf16 (1 cycle/row); top-2 selection in f32 (exact
    expert choice); I/O in bf16 halves HBM traffic.

Sharding: tokens (4*4096 = 16384) split evenly across 8 cores; A/B and
the small expert-expand matrix are replicated.
"""

import numpy as np
import ml_dtypes

N_CORES = 8
B_SZ, S_SZ, D = 4, 4096, 2048
E, R, C = 8, 16, 128
T_FULL = B_SZ * S_SZ            # 16384 tokens
T_CORE = T_FULL // N_CORES      # 2048 tokens per core
GROUP = 512                     # token group (matmul moving dim)
N_GROUPS = T_CORE // GROUP      # 4
N_SUB = GROUP // 128            # 4 sub-tiles of 128 tokens
KD = D // 128                   # 16 contraction chunks
SCALING = 2.0

_CACHE = {}


def _build_nc(split_waits=True):
    import concourse.bass as bass
    import concourse.tile as tile
    import concourse.mybir as mybir
    from contextlib import ExitStack

    f32 = mybir.dt.float32
    f32r = mybir.dt.float32r
    bf16 = mybir.dt.bfloat16

    nc = bass.Bass()
    h_d = nc.declare_dram_parameter("h", [T_CORE, D], bf16, isOutput=False)
    p_d = nc.declare_dram_parameter("p_L", [T_CORE, E], f32, isOutput=False)
    ab_d = nc.declare_dram_parameter("AB", [128, 2 * KD * 128], bf16, isOutput=False)
    m_d = nc.declare_dram_parameter("Mexp", [E, C], f32r, isOutput=False)
    i_d = nc.declare_dram_parameter("Ident", [128, 128], f32, isOutput=False)
    o_d = nc.declare_dram_parameter("out", [T_CORE, D], bf16, isOutput=True)

    AX = mybir.AxisListType
    OP = mybir.AluOpType

    with ExitStack() as ctx:
        tc = ctx.enter_context(tile.TileContext(nc))
        consts = ctx.enter_context(tc.tile_pool(name="consts", bufs=1))
        htpool = ctx.enter_context(tc.tile_pool(name="ht", bufs=1))
        gpool = ctx.enter_context(tc.tile_pool(name="gates", bufs=2))
        gsbpool = ctx.enter_context(tc.tile_pool(name="gsb", bufs=N_GROUPS))
        utspool = ctx.enter_context(tc.tile_pool(name="uts", bufs=2))
        outpool = ctx.enter_context(tc.tile_pool(name="osb", bufs=2))
        ps_u = ctx.enter_context(tc.tile_pool(name="ps_u", bufs=2, space="PSUM"))
        ps_g = ctx.enter_context(tc.tile_pool(name="ps_g", bufs=2, space="PSUM"))
        ps_o = ctx.enter_context(tc.tile_pool(name="ps_o", bufs=4, space="PSUM"))

        # ---- few, large DMAs: p on GpSimd (its only DMA); consts then
        # transposes on SP. Last 512-group split in two 256-token pieces
        # to shrink the post-DMA tail.
        p_all = consts.tile([128, N_GROUPS * N_SUB, E], f32)
        nc.gpsimd.dma_start(
            out=p_all, in_=p_d[:, :].rearrange("(q p) e -> p q e", p=128)
        )
        AB_sb = consts.tile([128, 2 * KD * 128], bf16)
        nc.sync.dma_start(out=AB_sb, in_=ab_d[:, :])
        M_sb = consts.tile([E, C], f32r)
        nc.sync.dma_start(out=M_sb, in_=m_d[:, :])
        I_sb = consts.tile([128, 128], f32)
        nc.sync.dma_start(out=I_sb, in_=i_d[:, :])
        A_sb = AB_sb[:, : KD * 128].rearrange("p (k c) -> p k c", k=KD)
        B_sb = AB_sb[:, KD * 128 :]

        PIECES = [(0, 512), (512, 512), (1024, 512), (1536, 512)]
        ht = []
        for t0, sz in PIECES:
            hth = htpool.tile([128, KD, sz], bf16, tag=f"ht{t0}")
            nc.sync.dma_start_transpose(out=hth, in_=h_d[t0 : t0 + sz, :])
            ht.append(hth)

        # ---- gate blocks for every group (only depend on p_L) ----
        G_tiles = []
        for g in range(N_GROUPS):
            p_sb = p_all[:, g * N_SUB : (g + 1) * N_SUB, :]
            m1 = gpool.tile([128, N_SUB, 1], f32, tag="m1")
            nc.vector.tensor_reduce(out=m1, in_=p_sb, axis=AX.X, op=OP.max)
            mlt = gpool.tile([128, N_SUB, E], f32, tag="mlt")
            nc.vector.tensor_tensor(
                out=mlt, in0=p_sb, in1=m1.broadcast_to([128, N_SUB, E]), op=OP.is_lt
            )
            pm = gpool.tile([128, N_SUB, E], f32, tag="pm")
            nc.vector.tensor_mul(pm, p_sb, mlt)
            m2 = gpool.tile([128, N_SUB, 1], f32, tag="m2")
            nc.vector.tensor_reduce(out=m2, in_=pm, axis=AX.X, op=OP.max)
            ge2 = gpool.tile([128, N_SUB, E], f32, tag="ge2")
            nc.vector.tensor_tensor(
                out=ge2, in0=p_sb, in1=m2.broadcast_to([128, N_SUB, E]), op=OP.is_ge
            )
            gts = gpool.tile([128, N_SUB, E], f32, tag="gts")
            nc.vector.tensor_mul(gts, p_sb, ge2)

            gt_ps = ps_g.tile([128, GROUP], f32, tag="g")
            for s in range(N_SUB):
                nc.tensor.transpose(
                    out=gt_ps[:E, s * 128 : (s + 1) * 128],
                    in_=gts[:, s, :],
                    identity=I_sb,
                )
            gt_sb = gpool.tile([E, GROUP], f32r, tag="gtsb")
            nc.scalar.copy(out=gt_sb, in_=gt_ps[:E, :])
            G_ps = ps_g.tile([128, GROUP], f32, tag="g")
            nc.tensor.matmul(G_ps, lhsT=M_sb, rhs=gt_sb, start=True, stop=True)
            G_sb = gsbpool.tile([128, GROUP], f32, tag="gsb")
            nc.scalar.copy(out=G_sb, in_=G_ps)
            G_tiles.append(G_sb)

        # ---- PE warm-up: keep the PE busy through the otherwise-idle
        # window between the gate blocks and the first transpose landing,
        # so the p-state ramp (full clock after 3us continuous busy) is
        # already done when stage 1 starts. Results are never read.
        for w in range(14):
            warm_ps = ps_g.tile([128, GROUP], f32, tag="g")
            nc.tensor.matmul(
                warm_ps,
                lhsT=A_sb[:, w % KD, :],
                rhs=B_sb[:, :GROUP],
                start=True,
                stop=True,
            )

        # ---- software-pipelined stage 1 / stage 2 (one piece apart) ----
        uts_tiles = [None] * len(PIECES)

        def stage1(i):
            t0, sz = PIECES[i]
            U_ps = ps_u.tile([128, GROUP], f32, tag="u")
            for k in range(KD):
                nc.tensor.matmul(
                    U_ps[:, :sz],
                    lhsT=A_sb[:, k, :],
                    rhs=ht[i][:, k, :],
                    start=(k == 0),
                    stop=(k == KD - 1),
                )
            uts = utspool.tile([128, GROUP], bf16, tag="uts")
            g, off = t0 // GROUP, t0 % GROUP
            nc.vector.tensor_tensor(
                out=uts[:, :sz],
                in0=U_ps[:, :sz],
                in1=G_tiles[g][:, off : off + sz],
                op=OP.mult,
            )
            uts_tiles[i] = uts

        def stage2(i):
            t0, sz = PIECES[i]
            uts = uts_tiles[i]
            o_sb = outpool.tile([128, N_SUB, D], bf16, tag="osb")
            for s in range(sz // 128):
                for j in range(4):
                    o_ps = ps_o.tile([128, 512], f32, tag="ops")
                    nc.tensor.matmul(
                        o_ps,
                        lhsT=uts[:, s * 128 : (s + 1) * 128],
                        rhs=B_sb[:, j * 512 : (j + 1) * 512],
                        start=True,
                        stop=True,
                    )
                    dst = o_sb[:, s, j * 512 : (j + 1) * 512]
                    if j % 2 == 0:
                        nc.vector.tensor_copy(out=dst, in_=o_ps)
                    else:
                        nc.scalar.copy(out=dst, in_=o_ps)
            nc.sync.dma_start(
                out=o_d[t0 : t0 + sz, :].rearrange("(s p) d -> p s d", p=128),
                in_=o_sb[:, : sz // 128, :],
            )

        for i in range(len(PIECES)):
            stage1(i)
            if i >= 1:
                stage2(i - 1)
        stage2(len(PIECES) - 1)

    if split_waits:
        _split_matmul_waits(nc)
    return nc


def _split_matmul_waits(nc, max_waits=1):
    """Walrus codegen allows only one sync-wait slot on self-loading
    (fp32/fp32r) Matmult instructions. Move surplus waits onto a no-op
    EventSemaphore inserted immediately before, same engine — identical
    semantics (waits still complete before the matmul dispatches)."""
    import concourse.mybir as mybir

    n = 0
    for f in nc.m.functions:
        for blk in f.blocks:
            insts = blk.instructions
            new_list = []
            changed = False
            for inst in insts:
                si = inst.sync_info
                if (
                    type(inst).__name__ != "InstEventSemaphore"
                    and si is not None
                    and si.on_wait
                    and len(si.on_wait) > max_waits
                ):
                    surplus = list(si.on_wait[:-max_waits])
                    keep = list(si.on_wait[-max_waits:])
                    # EventSemaphore carriers take at most 2 waits each
                    for i in range(0, len(surplus), 2):
                        n += 1
                        ev = mybir.InstEventSemaphore(
                            name=f"I-swsplit-{n}", ins=[], outs=[]
                        )
                        ev.engine = inst.engine
                        ev.sync_info = mybir.SyncInfo(
                            on_wait=surplus[i : i + 2], on_update=[]
                        )
                        new_list.append(ev)
                    inst.sync_info = mybir.SyncInfo(
                        on_wait=keep, on_update=list(si.on_update or [])
                    )
                    changed = True
                new_list.append(inst)
            if changed:
                blk.instructions = new_list
    return n


def _host_prep(h, p_L, A, B):
    """Shard tokens across cores; build replicated helper matrices."""
    bf16 = ml_dtypes.bfloat16
    h_flat = np.ascontiguousarray(
        np.asarray(h, dtype=np.float32).reshape(T_FULL, D)
    ).astype(bf16)
    p_flat = np.ascontiguousarray(np.asarray(p_L, dtype=np.float32).reshape(T_FULL, E))
    # A_cat[d, c] = SCALING * A[e, r, d]
    A_cat = (np.asarray(A, dtype=np.float32) * SCALING).transpose(2, 0, 1).reshape(D, C)
    # XBAR layout: ht[c, k, t] = h[t, 128k + c]  =>  A3[c, k, :] = A_cat[128k+c, :]
    A3 = np.ascontiguousarray(
        A_cat.reshape(KD, 128, C).transpose(1, 0, 2)
    ).astype(bf16)
    # B_cat[c, d] = B[e, d, r]
    B_cat = np.ascontiguousarray(
        np.asarray(B, dtype=np.float32).transpose(0, 2, 1).reshape(C, D)
    ).astype(bf16)
    AB = np.ascontiguousarray(
        np.concatenate([A3.reshape(128, KD * 128), B_cat], axis=1)
    )
    Mexp = np.zeros((E, C), dtype=np.float32)
    for e in range(E):
        Mexp[e, e * R : (e + 1) * R] = 1.0
    Ident = np.eye(128, dtype=np.float32)
    in_maps = []
    for i in range(N_CORES):
        sl = slice(i * T_CORE, (i + 1) * T_CORE)
        in_maps.append(
            {
                "h": h_flat[sl],
                "p_L": p_flat[sl],
                "AB": AB,
                "Mexp": Mexp,
                "Ident": Ident,
            }
        )
    return in_maps


def _get_nc():
    if "nc" not in _CACHE:
        _CACHE["nc"] = _build_nc()
    return _CACHE["nc"]


def kernel(h, p_L, A, B):
    from concourse.bass_utils import run_bass_kernel_spmd

    nc = _get_nc()
    in_maps = _host_prep(h, p_L, A, B)
    res = run_bass_kernel_spmd(nc, in_maps, core_ids=list(range(N_CORES)))
    out = np.concatenate(
        [np.asarray(res.results[i]["out"]) for i in range(N_CORES)], axis=0
    )
    return out.astype(np.float32).reshape(B_SZ, S_SZ, D)
